# revision 2
# baseline (speedup 1.0000x reference)
"""Bass/Trainium2 kernel for nn_Attn_13846974562399.

Computes, for the reference module:
    proj   = enc @ W^T + bias          # [S, B, H]
    scores = einsum('bh,sbh->bs', hidden[0], proj)
    attn   = softmax(scores, axis=1)   # -> [B, 1, S]

Algebraic restructure used here:
    scores[b, s] = q[b] . enc[s, b] + (hidden[0,b] . bias)
    with q = hidden[0] @ W.
The per-b constant (hidden . bias) is invariant under softmax over s, so it
is dropped.  q ([B, H], ~128 KB) is computed on the host in float64; the
memory-bound work -- streaming the 268 MB encoder tensor and the batched
dot-products -- runs on 8 NeuronCores, data-parallel over the batch dim
(4 batches per core).

Per-core device program:
  - DMA q-shard broadcast to all 128 partitions ([128, 4*1024] in SBUF).
  - 16 tile loads of 2 MB each (fully contiguous; host pre-permutes the
    shard to [t, p, b, h] with s = p*16 + t).
  - 64 fused DVE tensor_tensor_reduce ops: scores[p, b, t] = sum_h
    enc_tile[p, b, h] * q[b, h].
  - Softmax over the 2048 scores per b: per-partition max (DVE) ->
    cross-partition max (GPSIMD all-reduce) -> exp with per-partition bias
    and fused free-dim sum (ACT) -> cross-partition sum (GPSIMD) ->
    reciprocal + scale (DVE) -> one 32 KB DMA out.
"""

import numpy as np

import concourse.bacc as bacc
import concourse.bass as bass
import concourse.mybir as mybir
import concourse.tile as tile
from concourse.bass_isa import ReduceOp
from concourse.bass_utils import run_bass_kernel_spmd

S, B, H = 2048, 32, 1024
NCORES = 8
BL = B // NCORES          # 4 local batches per core
P = 128                   # SBUF partitions
NT = S // P               # 16 s-tiles; s = p*NT + t
F32 = mybir.dt.float32

ENC_BUFS = 4              # in-flight 2 MB encoder tiles

# Populated by the most recent kernel() call (for test harnesses).
LAST_RESULTS = None
TRACE = False

_NC = None


def _build_bass():
    nc = bacc.Bacc()
    enc = nc.dram_tensor("enc", [NT, P, BL, H], F32, kind="ExternalInput")
    q = nc.dram_tensor("q", [BL, H], F32, kind="ExternalInput")
    out = nc.dram_tensor("attn", [P, BL, NT], F32, kind="ExternalOutput")

    with tile.TileContext(nc) as tc:
        with (
            tc.tile_pool(name="encp", bufs=ENC_BUFS) as enc_pool,
            tc.tile_pool(name="small", bufs=1) as small,
        ):
            # q broadcast across all partitions: qb[p, b, h] = q[b, h]
            qb = small.tile([P, BL, H], F32)
            q_ap = q.ap()
            q_bcast_src = bass.AP(
                tensor=q_ap.tensor,
                offset=q_ap.offset,
                ap=[[0, P], q_ap.ap[0], q_ap.ap[1]],
            )
            nc.gpsimd.dma_start(out=qb, in_=q_bcast_src)

            scores = small.tile([P, BL, NT], F32)
            dummy = small.tile([P, 1], F32)
            enc_ap = enc.ap()

            for t in range(NT):
                et = enc_pool.tile([P, BL, H], F32)
                nc.sync.dma_start(out=et, in_=enc_ap[t])
                for b in range(BL):
                    # out = (in0 * 1.0) * in1; accum_out = sum(out) over h.
                    # (InstTensorScalarPtr: TENSOR_TENSOR_REDUCE crashes this
                    # runtime's NX ucode, scalar_tensor_tensor is equivalent.)
                    nc.vector.scalar_tensor_tensor(
                        out=dummy.broadcast_to((P, H)),
                        in0=et[:, b, :],
                        scalar=1.0,
                        in1=qb[:, b, :],
                        op0=mybir.AluOpType.mult,
                        op1=mybir.AluOpType.mult,
                        accum_out=scores[:, b, t : t + 1],
                    )

            # softmax over (p, t) for each b
            m = small.tile([P, BL], F32)
            nc.vector.tensor_reduce(
                out=m, in_=scores, axis=mybir.AxisListType.X, op=mybir.AluOpType.max
            )
            nc.gpsimd.partition_all_reduce(m, m, P, ReduceOp.max)
            negm = small.tile([P, BL], F32)
            nc.vector.tensor_scalar_mul(out=negm, in0=m, scalar1=-1.0)

            e = small.tile([P, BL, NT], F32)
            ssum = small.tile([P, BL], F32)
            for b in range(BL):
                nc.scalar.activation(
                    out=e[:, b, :],
                    in_=scores[:, b, :],
                    func=mybir.ActivationFunctionType.Exp,
                    bias=negm[:, b : b + 1],
                    scale=1.0,
                    accum_out=ssum[:, b : b + 1],
                )
            nc.gpsimd.partition_all_reduce(ssum, ssum, P, ReduceOp.add)
            rz = small.tile([P, BL], F32)
            nc.vector.reciprocal(rz, ssum)

            attn_sb = small.tile([P, BL, NT], F32)
            for b in range(BL):
                nc.vector.tensor_scalar_mul(
                    out=attn_sb[:, b, :], in0=e[:, b, :], scalar1=rz[:, b : b + 1]
                )
            nc.sync.dma_start(out=out.ap(), in_=attn_sb)

    nc.compile()
    return nc


def kernel(hidden, encoder_outputs, W, b):
    global _NC, LAST_RESULTS
    hidden = np.asarray(hidden, dtype=np.float32)
    enc = np.asarray(encoder_outputs, dtype=np.float32)
    W = np.asarray(W, dtype=np.float32)

    # q = hidden[0] @ W  (fp64 accumulate on host; tiny vs the 268 MB stream).
    # The bias term contributes a per-b constant to the scores, which softmax
    # cancels, so `b` is unused.
    q_full = (hidden[0].astype(np.float64) @ W.astype(np.float64)).astype(np.float32)

    in_maps = []
    for c in range(NCORES):
        enc_c = enc[:, BL * c : BL * (c + 1), :]            # [S, BL, H]
        # [t, p, b, h] with s = p*NT + t
        enc_r = np.ascontiguousarray(
            enc_c.reshape(P, NT, BL, H).transpose(1, 0, 2, 3)
        )
        in_maps.append(
            {
                "enc": enc_r,
                "q": np.ascontiguousarray(q_full[BL * c : BL * (c + 1)]),
            }
        )

    if _NC is None:
        _NC = _build_bass()

    LAST_RESULTS = run_bass_kernel_spmd(
        _NC, in_maps, core_ids=list(range(NCORES)), trace=TRACE
    )

    out = np.empty((B, 1, S), dtype=np.float32)
    for c in range(NCORES):
        a = LAST_RESULTS.results[c]["attn"]                 # [P, BL, NT]
        out[BL * c : BL * (c + 1), 0, :] = a.transpose(1, 0, 2).reshape(BL, S)
    return out
